# revision 21
# baseline (speedup 1.0000x reference)
"""LeNet C3 grouped-conv layer as a Trainium2 Bass/Tile kernel.

Math: y[b,o,h,w] = sum_{c,dy,dx} W[o,c,dy,dx] * x[b,c,h+dy,w+dx] + bias[o]
with W the dense 16x6x5x5 weight built from the C3 connectivity tables
(absent connections are zero).

Mapping (per core, 16 images of the batch, processed as 4 quads of 4):
  - DRAM layouts are chosen so every block moves with ONE 3-dim DMA:
      x_dev[q, c, h, img, w]  bf16  (host pre-transposed)
      y_dev[q, o, h, img, w]  bf16, w padded to 256 (host strips pad)
  - Input tile per block: 20 consecutive rows x 6 channels x 4 images,
    partitions p = c*20 + dr, cols = img*256 + w.  K = 120.  One SWDGE
    DMA: 120 descriptors of 2 KB.
  - Each 20-row block yields 16 output rows via two INTERLEAVED window
    phases: partition s of phase ph covers output row h0 + 2*s + ph, so
    a partition's two phase results are ADJACENT output rows and the
    whole block's output is one contiguous 4 KB run per partition:
    ot cols = ph*1024 + img*256 + w.  One HWDGE DMA per block: 128
    descriptors of 4 KB.  Phase selection lives in the stationary lhsT:
    lhsT_ph[(c,dr), (o,s)] = W[o, c, dr-2*s-ph, dx] (zero outside band).
  - 5 matmuls (dx = 0..4) accumulate in PSUM with the moving AP shifted
    by dx cols; images processed in pairs: N = 508 bf16 (1 cycle/col).
    Out cols 0..251 img A, 256..507 img B, 252..255 garbage seam
    (becomes output pad).
  - Bias is added during the PSUM->SBUF copy (DVE tensor_scalar_add,
    f32 -> bf16 cast).  The SP and Activation engines are reserved for
    issuing output DMAs: a dma_start's semaphore wait blocks the issuing
    engine's sequencer, so the three DMA-capable paths (SP, Activation
    HWDGE + gpsimd SWDGE) rotate over the per-block output DMAs while
    gpsimd also carries the input stream.  This 3-way rotation is what
    lets the per-block pipeline run at the PE's matmul streaming rate:
    measured 245 us/iteration steady-state = the pure-matmul floor
    (probe_pe.py), vs 533 us for the previous f32r 8-DMA-per-block
    version.
  - Blocks: h0 = 0,16,...,224 produce rows 0..239; a final block at
    h0 = 236 produces rows 240..251 with compact M = 96 variants
    (p = o*6 + s', row 240 + 2*s' + ph).
"""

import os
import sys

sys.path.insert(0, "/opt/trn_rl_repo")

import numpy as np

_CH3 = np.array([[0, 1, 2], [1, 2, 3], [2, 3, 4], [3, 4, 5], [0, 4, 5], [0, 1, 5]])
_CH4 = np.array(
    [
        [0, 1, 2, 3],
        [1, 2, 3, 4],
        [2, 3, 4, 5],
        [0, 3, 4, 5],
        [0, 1, 4, 5],
        [0, 1, 2, 5],
        [0, 1, 3, 4],
        [1, 2, 4, 5],
        [0, 2, 3, 5],
    ]
)
_CH6 = np.array([[0, 1, 2, 3, 4, 5]])

_B_PER_CORE = 16  # 128 batch / 8 cores
_N_CORES = 8
_H = 256
_W = 256
_HO = 252
_WO = 252
_WP = 256  # padded output row length (bf16 rows -> 512B runs)
_R = 20  # input rows per block
_K = 6 * _R  # 120 contraction partitions

_module_cache = {}


def _to_bf16(a):
    import ml_dtypes

    return np.ascontiguousarray(np.asarray(a, np.float32).astype(ml_dtypes.bfloat16))


def _dense_weights(w3, b3, w4, b4, w6, b6):
    W = np.zeros((16, 6, 5, 5), np.float32)
    bias = np.zeros((16,), np.float32)
    for i in range(6):
        W[i, _CH3[i]] = w3[i]
    bias[0:6] = b3
    for i in range(9):
        W[6 + i, _CH4[i]] = w4[i]
    bias[6:15] = b4
    W[15, _CH6[0]] = w6[0]
    bias[15] = np.asarray(b6).reshape(-1)[0]
    return W, bias


def _host_tensors(w3, b3, w4, b4, w6, b6):
    W, bias = _dense_weights(w3, b3, w4, b4, w6, b6)
    # regular blocks: lhsT[(c, dr), (ph, dx, o, s)] = W[o, c, dr-2s-ph, dx]
    lhsT = np.zeros((6, _R, 2, 5, 16, 8), np.float32)
    for dr in range(_R):
        for ph in range(2):
            for s in range(8):
                dy = dr - 2 * s - ph
                if 0 <= dy < 5:
                    # [c, dx, o] <- W[o, c, dy, dx]
                    lhsT[:, dr, ph, :, :, s] = W[:, :, dy, :].transpose(1, 2, 0)
    lhsT = np.ascontiguousarray(lhsT.reshape(_K, 2 * 5 * 128))
    # final block (h0=236): rows 240 + 2s' + ph, s' in [0,6), p = o*6+s'.
    lhsTf = np.zeros((6, _R, 2, 5, 16, 6), np.float32)
    for dr in range(_R):
        for ph in range(2):
            for s in range(6):
                dy = dr - (4 + 2 * s + ph)
                if 0 <= dy < 5:
                    lhsTf[:, dr, ph, :, :, s] = W[:, :, dy, :].transpose(1, 2, 0)
    lhsTf = np.ascontiguousarray(lhsTf.reshape(_K, 2 * 5 * 96))
    biasf = np.repeat(bias, 8).reshape(128, 1).astype(np.float32)  # p = o*8+s
    biasq = np.repeat(bias, 6).reshape(96, 1).astype(np.float32)  # p = o*6+s'
    return lhsT, lhsTf, biasf, biasq


def _build_module(reps=1):
    if ("nc", reps) in _module_cache:
        return _module_cache[("nc", reps)]

    import concourse.bacc as bacc
    import concourse.mybir as mybir
    from concourse.tile import TileContext

    f32 = mybir.dt.float32
    bf16 = mybir.dt.bfloat16

    nc = bacc.Bacc("TRN2", target_bir_lowering=False, debug=False)
    x = nc.dram_tensor(
        "x", [_B_PER_CORE // 4, 6, _H, 4, _W], bf16, kind="ExternalInput"
    ).ap()
    lhsT = nc.dram_tensor("lhsT", [_K, 1280], bf16, kind="ExternalInput").ap()
    lhsTf = nc.dram_tensor("lhsTf", [_K, 960], bf16, kind="ExternalInput").ap()
    biasf = nc.dram_tensor("biasf", [128, 1], f32, kind="ExternalInput").ap()
    biasq = nc.dram_tensor("biasq", [96, 1], f32, kind="ExternalInput").ap()
    y = nc.dram_tensor(
        "y", [_B_PER_CORE // 4, 16, _HO, 4, _WP], bf16, kind="ExternalOutput"
    ).ap()

    n_blk = 16  # 15 blocks at h0=16k + final block at h0=236

    with TileContext(nc) as tc:
        with (
            tc.tile_pool(name="const", bufs=1) as cpool,
            tc.tile_pool(name="xin", bufs=10) as xpool,
            tc.tile_pool(name="oup", bufs=8) as opool,
            tc.tile_pool(name="psum", bufs=8, space="PSUM") as ppool,
        ):
            wt = cpool.tile([_K, 1280], bf16)
            nc.sync.dma_start(out=wt, in_=lhsT)
            wtf = cpool.tile([_K, 960], bf16)
            nc.sync.dma_start(out=wtf, in_=lhsTf)
            bf = cpool.tile([128, 1], f32)
            nc.sync.dma_start(out=bf, in_=biasf)
            bq = cpool.tile([96, 1], f32)
            nc.sync.dma_start(out=bq, in_=biasq)

            for rep in range(reps):
              for quad in range(_B_PER_CORE // 4):
                for blk in range(n_blk):
                    final = blk == n_blk - 1
                    h0 = 16 * blk if not final else 236
                    M = 96 if final else 128
                    xt = xpool.tile([_K, 1024], bf16)
                    # p = c*20 + dr, col = img*256 + w.  One SWDGE DMA:
                    # src (c, r*img*w) merges to 3 dims, 2 KB descriptors.
                    nc.gpsimd.dma_start(
                        out=xt, in_=x[quad][:, h0 : h0 + _R, :, :]
                    )
                    # col = ph*1024 + img*256 + w
                    ot = opool.tile([128, 2048], bf16)
                    for ph in range(2):
                        for g in range(2):
                            ps = ppool.tile([128, 508], f32)
                            for dx in range(5):
                                if final:
                                    lw = wtf[:, (ph * 5 + dx) * 96 : (ph * 5 + dx + 1) * 96]
                                else:
                                    lw = wt[:, (ph * 5 + dx) * 128 : (ph * 5 + dx + 1) * 128]
                                nc.tensor.matmul(
                                    ps[0:M, :],
                                    lw,
                                    xt[:, 512 * g + dx : 512 * g + dx + 508],
                                    start=(dx == 0),
                                    stop=(dx == 4),
                                )
                            # PSUM -> SBUF with bias add, f32 -> bf16 cast.
                            dst = ot[
                                0:M,
                                ph * 1024 + 512 * g : ph * 1024 + 512 * g + 508,
                            ]
                            bias_ap = bq if final else bf
                            nc.vector.tensor_scalar_add(dst, ps[0:M, :], bias_ap)
                    # One HWDGE DMA per block: partition (o,s) row
                    # h0 + 2s + ph, contiguous (ph, img, w) 4 KB run.
                    oeng = (nc.sync, nc.scalar, nc.gpsimd)[blk % 3]
                    if final:
                        dest = y[quad][:, 240:252, :, :].rearrange(
                            "o (s ph) img w -> o s (ph img w)", s=6, ph=2
                        )
                        oeng.dma_start(out=dest, in_=ot[0:96, :])
                    else:
                        dest = y[quad][:, h0 : h0 + 16, :, :].rearrange(
                            "o (s ph) img w -> o s (ph img w)", s=8, ph=2
                        )
                        oeng.dma_start(out=dest, in_=ot)

    nc.compile()
    _module_cache[("nc", reps)] = nc
    return nc


def _run(inputs, trace=False):
    # The NTFF trace hook (antenv.axon_hooks) does not exist in this
    # container; a stray BASS_TRACE=1 in the environment would crash the
    # axon redirect path. Force tracing off unless explicitly requested.
    if not trace:
        os.environ["BASS_NEVER_TRACE"] = "1"

    from concourse.bass_utils import run_bass_kernel_spmd

    # x: [128, 6, 256, 256] f32 -> per core [4 quad, 6, 256, 4 img, 256] bf16
    x = _to_bf16(inputs["x"])
    xr = np.ascontiguousarray(
        x.reshape(_N_CORES, 4, 4, 6, _H, _W).transpose(0, 1, 3, 4, 2, 5)
    )
    lhsT, lhsTf, biasf, biasq = _host_tensors(
        np.asarray(inputs["w3"], np.float32),
        np.asarray(inputs["b3"], np.float32),
        np.asarray(inputs["w4"], np.float32),
        np.asarray(inputs["b4"], np.float32),
        np.asarray(inputs["w6"], np.float32),
        np.asarray(inputs["b6"], np.float32),
    )
    lhsT = _to_bf16(lhsT)
    lhsTf = _to_bf16(lhsTf)
    nc = _build_module()
    in_maps = [
        {
            "x": xr[i],
            "lhsT": lhsT,
            "lhsTf": lhsTf,
            "biasf": biasf,
            "biasq": biasq,
        }
        for i in range(_N_CORES)
    ]
    res = run_bass_kernel_spmd(
        nc, in_maps, core_ids=list(range(_N_CORES)), trace=trace
    )
    # y_dev [4, 16, 252, 4, 256] bf16 -> [16, 16, 252, 252] f32 per core
    out = np.empty((128, 16, _HO, _WO), np.float32)
    for i in range(_N_CORES):
        yd = np.asarray(res.results[i]["y"])  # [4, 16, 252, 4, 256]
        out[16 * i : 16 * (i + 1)] = (
            yd[:, :, :, :, :_WO].astype(np.float32).transpose(0, 3, 1, 2, 4)
        ).reshape(16, 16, _HO, _WO)
    return out, res


def kernel(**inputs):
    out, _ = _run(inputs, trace=False)
    return out
